# revision 1
# baseline (speedup 1.0000x reference)
"""Trainium2 Bass kernel for a 3-layer edge-conditioned GNN (ESAGEConv-like)
with global-add-pool readout, distributed across 8 NeuronCores.

Algorithm (algebraic restructuring of the reference):
    msg  = concat(x[src], ea) @ We + be
    aggr = segment_sum(msg, dst)
         = segment_sum(x[src], dst) @ We_x + segment_sum([ea|1], dst) @ [We_e;be]
so the edge-level matmul collapses to node-level matmuls plus one sparse
aggregation g = A @ x per layer.  h = segment_sum([ea|1], dst) is
layer-independent and computed once (its ones-column yields the degree row,
which applies `be` exactly).

Distribution: nodes are sharded by graph id into 8 contiguous ranges (graphs
are contiguous because `batch` is sorted).  Each core owns the edges whose dst
lands in its range, keeps a full padded replica of x in HBM for gathers, and
all-gathers its computed node block after each of the first two layers.  The
final pooled readout is computed locally per core (8 graphs per core).

Per-core pipeline, per 128-node dst window:
  - dma_gather (SWDGE, 4 queues) fetches x[src] rows (512B fp32) for the
    window's edges, 128 edges per SBUF tile [128e, 128f],
  - a one-hot matrix [128e, 128n] is built on DVE from window-local dst ids
    via tensor_tensor(is_equal) against an iota row,
  - PE matmuls accumulate g^T[f, n] += gathered^T @ onehot in PSUM,
  - node phase: out^T = We_x^T g^T + We_e^T h^T + Wr^T x^T, ReLU+bias on ACT,
  - PE transpose -> replica write (layers 0,1) or pooling matmul (layer 2).
"""

import numpy as np

P = 128          # partitions / dst-window size
MAX_IDX_PER_CALL = 2048   # needs dynamic_dma_scratch_size >= 32768
TILES_PER_CALL = MAX_IDX_PER_CALL // P
OH_GROUP = 4     # one-hot tiles built per DVE instruction
N_CORES = 8


def _plan(x, edge_index, edge_attr, batch, n_graphs):
    """Host-side preprocessing: shard, sort, pad, and pack all per-core data."""
    N, Din = x.shape
    E = edge_index.shape[1]
    De = edge_attr.shape[1]
    EA_W = ((De + 1 + 7) // 8) * 8          # ea features + ones column, padded

    src = np.asarray(edge_index[0], dtype=np.int64)
    dst = np.asarray(edge_index[1], dtype=np.int64)
    batch = np.asarray(batch, dtype=np.int64)

    g_per_core = n_graphs // N_CORES
    # graph -> node count; device node ranges (batch is sorted)
    graph_sizes = np.bincount(batch, minlength=n_graphs)
    g_starts = np.concatenate([[0], np.cumsum(graph_sizes)])
    s_d = np.array([g_starts[d * g_per_core] for d in range(N_CORES)])
    e_d = np.array([g_starts[(d + 1) * g_per_core] for d in range(N_CORES)])
    n_d = e_d - s_d

    NODES_PAD = int(np.ceil(n_d.max() / P) * P)
    NW = NODES_PAD // P
    TABLE_ROWS = N_CORES * NODES_PAD
    SPLIT = TABLE_ROWS // 2
    assert SPLIT <= 32767 and TABLE_ROWS - SPLIT <= 32767, TABLE_ROWS

    node_dev = np.repeat(np.arange(N_CORES), n_d)            # [N]
    rowof = (node_dev * NODES_PAD + (np.arange(N) - s_d[node_dev])).astype(np.int64)

    # per-edge device / window / slot / half / table idx
    edev = node_dev[dst]
    dloc = dst - s_d[edev]
    ewin = dloc // P
    eslot = dloc % P
    srow = rowof[src]
    ehalf = (srow >= SPLIT).astype(np.int64)
    eidx16 = (srow - ehalf * SPLIT).astype(np.int64)

    # group sizes per (dev, win, half)
    key = (edev * NW + ewin) * 2 + ehalf
    counts = np.bincount(key, minlength=N_CORES * NW * 2).reshape(N_CORES, NW, 2)
    T_wh = np.ceil(counts.max(axis=0) / P).astype(np.int64)   # [NW, 2] tiles
    NT = int(T_wh.sum())
    E_PAD = NT * P

    # stream tile offsets per (win, half): all lo groups first, then all hi
    # groups, so each half is one contiguous gather stream (full-size calls).
    tile_off = np.zeros((NW, 2), dtype=np.int64)
    acc = 0
    for h in range(2):
        for w in range(NW):
            tile_off[w, h] = acc
            acc += T_wh[w, h]
    lo_tiles = int(T_wh[:, 0].sum())

    order = np.argsort(key, kind="stable")

    per_core = []
    for d in range(N_CORES):
        sel = order[edev[order] == d]
        cnt_d = counts[d]                                   # [NW, 2]
        # stream position of each edge of this device
        pos_in_grp = np.concatenate([np.arange(c) for c in cnt_d.reshape(-1)]) \
            if sel.size else np.zeros(0, np.int64)
        grp_base = (tile_off[ewin[sel], ehalf[sel]] * P)
        pos = grp_base + pos_in_grp

        idx_stream = np.zeros(E_PAD, dtype=np.int16)
        dloc_stream = np.full(E_PAD, -1000.0, dtype=np.float16)
        ea_stream = np.zeros((E_PAD, EA_W), dtype=np.float16)
        idx_stream[pos] = eidx16[sel].astype(np.int16)
        dloc_stream[pos] = eslot[sel].astype(np.float16)
        ea_stream[pos, :De] = np.asarray(edge_attr, dtype=np.float16)[sel]
        ea_stream[pos, De] = 1.0

        # pack layouts
        idx_pack = np.tile(idx_stream.reshape(E_PAD // 16, 16).T, (8, 1))  # [128, E/16]
        dloc_pack = dloc_stream.reshape(NT, P).T.copy()                    # [128, NT]
        ea_pack = ea_stream.reshape(NT, P, EA_W).transpose(1, 0, 2).copy() # [128, NT, EA_W]

        gid = np.full((P, NW), -1.0, dtype=np.float16)
        loc = np.arange(n_d[d])
        lg = batch[s_d[d]:e_d[d]] - d * g_per_core
        gid[loc % P, loc // P] = lg.astype(np.float16)

        x0T = np.zeros((P, NODES_PAD), dtype=np.float32)
        x0T[:Din, :n_d[d]] = np.asarray(x[s_d[d]:e_d[d]], dtype=np.float32).T

        per_core.append(dict(idx=idx_pack, dloc=dloc_pack, ea=ea_pack,
                             gid=gid, x0T=x0T))

    xrep0 = np.zeros((TABLE_ROWS, P), dtype=np.float16)
    rows = rowof
    xrep0[rows, :Din] = np.asarray(x, dtype=np.float16)

    meta = dict(N=N, Din=Din, E=E, De=De, EA_W=EA_W, NODES_PAD=NODES_PAD,
                NW=NW, TABLE_ROWS=TABLE_ROWS, SPLIT=SPLIT, NT=NT, E_PAD=E_PAD,
                T_wh=T_wh, tile_off=tile_off, g_per_core=g_per_core,
                lo_tiles=lo_tiles)
    return meta, per_core, xrep0


def _build(meta, weights):
    """Build the SPMD Bass program (identical on all 8 cores)."""
    import concourse.bass as bass
    import concourse.mybir as mybir
    from concourse import bacc
    from concourse.tile import TileContext

    f32 = mybir.dt.float32
    f16 = mybir.dt.float16
    NODES_PAD, NW, EA_W = meta["NODES_PAD"], meta["NW"], meta["EA_W"]
    TABLE_ROWS, SPLIT, NT = meta["TABLE_ROWS"], meta["SPLIT"], meta["NT"]
    T_wh, tile_off = meta["T_wh"], meta["tile_off"]
    LO_TILES = meta["lo_tiles"]
    GPC = meta["g_per_core"]
    n_layers = len(weights)
    OUT = weights[0]["Wx"].shape[1]

    nc = bacc.Bacc(num_devices=N_CORES, num_swdge_queues=4,
                   dynamic_dma_scratch_size=32768)

    xrep0_d = nc.dram_tensor("xrep0", (TABLE_ROWS, P), f16, kind="ExternalInput")
    x0T_d = nc.dram_tensor("x0T", (P, NODES_PAD), f32, kind="ExternalInput")
    idx_d = nc.dram_tensor("idxs", (P, meta["E_PAD"] // 16), mybir.dt.int16,
                           kind="ExternalInput")
    dloc_d = nc.dram_tensor("dloc", (P, NT), f16, kind="ExternalInput")
    ea_d = nc.dram_tensor("ea", (P, NT, EA_W), f16, kind="ExternalInput")
    gid_d = nc.dram_tensor("gid", (P, NW), f16, kind="ExternalInput")
    iota_d = nc.dram_tensor("iota", (P, P), f16, kind="ExternalInput")
    ident_d = nc.dram_tensor("ident", (P, P), f32, kind="ExternalInput")
    w_d = []
    for l in range(n_layers):
        w_d.append(dict(
            Wx=nc.dram_tensor(f"Wx{l}", (P, OUT), f32, kind="ExternalInput"),
            We=nc.dram_tensor(f"We{l}", (EA_W, OUT), f32, kind="ExternalInput"),
            Wr=nc.dram_tensor(f"Wr{l}", (P, OUT), f32, kind="ExternalInput"),
            br=nc.dram_tensor(f"br{l}", (OUT, 1), f32, kind="ExternalInput"),
        ))
    out_d = nc.dram_tensor("out", (GPC, OUT), f32, kind="ExternalOutput")

    agin = [nc.dram_tensor(f"agin{l}", (NODES_PAD, P), f16, kind="Internal")
            for l in range(n_layers - 1)]
    repl = [nc.dram_tensor(f"rep{l}", (TABLE_ROWS, P), f16, kind="Internal",
                           addr_space="Shared")
            for l in range(n_layers - 1)]
    rep_groups = [list(range(N_CORES))]

    with TileContext(nc) as tc:
        from contextlib import ExitStack
        ctx = ExitStack()
        with ctx:
            const = ctx.enter_context(tc.tile_pool(name="const", bufs=1))
            gpool = ctx.enter_context(tc.tile_pool(name="gather", bufs=6))
            ohpool = ctx.enter_context(tc.tile_pool(name="oh", bufs=6))
            eapool = ctx.enter_context(tc.tile_pool(name="eat", bufs=2))
            npool = ctx.enter_context(tc.tile_pool(name="nodes", bufs=3))
            xpool_a = ctx.enter_context(tc.tile_pool(name="xta", bufs=1))
            xpool_b = ctx.enter_context(tc.tile_pool(name="xtb", bufs=1))
            hpool = ctx.enter_context(tc.tile_pool(name="ht", bufs=1))
            ps_g = ctx.enter_context(tc.tile_pool(name="psg", bufs=2, space="PSUM"))
            ps_h = ctx.enter_context(tc.tile_pool(name="psh", bufs=1, space="PSUM"))
            ps_o = ctx.enter_context(tc.tile_pool(name="pso", bufs=2, space="PSUM"))
            ps_t = ctx.enter_context(tc.tile_pool(name="pst", bufs=1, space="PSUM"))
            ps_p = ctx.enter_context(tc.tile_pool(name="psp", bufs=1, space="PSUM"))

            # ---- persistent loads -------------------------------------------
            idx_sb = const.tile([P, meta["E_PAD"] // 16], mybir.dt.int16)
            nc.sync.dma_start(idx_sb[:, :], idx_d[:, :])
            dloc_sb = const.tile([P, NT], f16)
            nc.sync.dma_start(dloc_sb[:, :], dloc_d[:, :])
            gid_sb = const.tile([P, NW], f16)
            nc.sync.dma_start(gid_sb[:, :], gid_d[:, :])
            iota_sb = const.tile([P, P], f16)
            nc.sync.dma_start(iota_sb[:, :], iota_d[:, :])
            ident_sb = const.tile([P, P], f32)
            nc.sync.dma_start(ident_sb[:, :], ident_d[:, :])
            w_sb = []
            for l in range(n_layers):
                w_sb.append(dict(
                    Wx=const.tile([P, OUT], f32, tag=f"wx{l}", name=f"wx{l}"),
                    We=const.tile([EA_W, OUT], f32, tag=f"we{l}", name=f"we{l}"),
                    Wr=const.tile([P, OUT], f32, tag=f"wr{l}", name=f"wr{l}"),
                    br=const.tile([OUT, 1], f32, tag=f"br{l}", name=f"brt{l}"),
                ))
                for k in ("Wx", "We", "Wr", "br"):
                    nc.sync.dma_start(w_sb[l][k][:, :], w_d[l][k][:, :])

            xt_a = xpool_a.tile([P, NODES_PAD], f32)
            xt_b = xpool_b.tile([P, NODES_PAD], f32)
            nc.sync.dma_start(xt_a[:, :], x0T_d[:, :])
            hT = hpool.tile([EA_W, NODES_PAD], f32)

            acc_sb = const.tile([GPC, OUT], f32)
            nc.vector.memset(acc_sb[:, :], 0.0)

            qrr = [0]  # gather queue round-robin
            T_MAX = max(1, int(T_wh.sum(axis=1).max()))
            HI_TILES = NT - LO_TILES

            # gather-call state, reset per layer: each half of the edge stream
            # is one contiguous run of full-size dma_gather calls, emitted
            # lazily as windows consume their tiles.
            gcalls = {}

            def gather_call(l, half, c):
                """Emit (memoized) gather call c of the given half-stream."""
                key = (half, c)
                if key in gcalls:
                    return gcalls[key]
                src_tbl = xrep0_d if l == 0 else repl[l - 1]
                tbl = src_tbl[0:SPLIT, :] if half == 0 else src_tbl[SPLIT:TABLE_ROWS, :]
                h_tiles = LO_TILES if half == 0 else HI_TILES
                h_off = 0 if half == 0 else LO_TILES
                t0 = c * TILES_PER_CALL
                n_t = min(TILES_PER_CALL, h_tiles - t0)
                dest = gpool.tile([P, TILES_PER_CALL, P], f16, tag="gd",
                                  name="gdest")
                a = (h_off + t0) * P
                nc.gpsimd.dma_gather(
                    dest[:, :n_t, :], tbl,
                    idx_sb[:, a // 16:(a + n_t * P) // 16],
                    n_t * P, n_t * P, P,
                    single_packet=False,
                    queue_num=qrr[0],
                )
                qrr[0] = (qrr[0] + 1) % 4
                gcalls[key] = dest
                return dest

            def stream_tile(l, st):
                """AP of gathered tile at stream tile index st."""
                half = 0 if st < LO_TILES else 1
                t = st if half == 0 else st - LO_TILES
                dest = gather_call(l, half, t // TILES_PER_CALL)
                return dest[:, t % TILES_PER_CALL, :]

            def window_tiles(l, w, psum_g, psum_h):
                """Gather + one-hot + aggregation matmuls for window w."""
                t_lo, t_hi = int(T_wh[w, 0]), int(T_wh[w, 1])
                t_tot = t_lo + t_hi
                if t_tot == 0:
                    return None
                # stream tile indices of this window (lo run + hi run)
                sts = (list(range(int(tile_off[w, 0]), int(tile_off[w, 0]) + t_lo))
                       + list(range(int(tile_off[w, 1]), int(tile_off[w, 1]) + t_hi)))

                if l == 0:
                    eat = eapool.tile([P, T_MAX, EA_W], f16, tag="ea", name="eat")
                    if t_lo:
                        nc.sync.dma_start(eat[:, :t_lo, :],
                                          ea_d[:, sts[0]:sts[0] + t_lo, :])
                    if t_hi:
                        nc.sync.dma_start(eat[:, t_lo:t_tot, :],
                                          ea_d[:, sts[t_lo]:sts[t_lo] + t_hi, :])

                # one-hots: one DVE op per OH_GROUP consecutive stream tiles
                oh_of = {}
                for seg0, seg_n in ((0, t_lo), (t_lo, t_hi)):
                    for g0 in range(0, seg_n, OH_GROUP):
                        gn = min(OH_GROUP, seg_n - g0)
                        st0 = sts[seg0 + g0]
                        oh = ohpool.tile([P, OH_GROUP, P], f16, tag="oh", name="oht")
                        nc.vector.tensor_tensor(
                            oh[:, :gn, :],
                            iota_sb[:, :].unsqueeze(1).broadcast_to([P, gn, P]),
                            dloc_sb[:, st0:st0 + gn]
                                .unsqueeze(2).broadcast_to([P, gn, P]),
                            mybir.AluOpType.is_equal,
                        )
                        for j in range(gn):
                            oh_of[seg0 + g0 + j] = oh[:, j, :]

                for i, st in enumerate(sts):
                    nc.tensor.matmul(psum_g[:, :], stream_tile(l, st), oh_of[i],
                                     start=(i == 0), stop=(i == t_tot - 1))
                if l == 0:
                    for i in range(t_tot):
                        nc.tensor.matmul(psum_h[:, :], eat[:, i, :], oh_of[i],
                                         start=(i == 0), stop=(i == t_tot - 1))
                return True

            for l in range(n_layers):
                gcalls.clear()
                xt_cur = xt_a if l % 2 == 0 else xt_b
                xt_next = xt_b if l % 2 == 0 else xt_a
                wl = w_sb[l]
                for w in range(NW):
                    ws = slice(w * P, (w + 1) * P)
                    psum_g = ps_g.tile([P, P], f32, tag="g")
                    psum_h = (ps_h.tile([EA_W, P], f32, tag="h", name="psum_h")
                              if l == 0 else None)
                    got = window_tiles(l, w, psum_g, psum_h)

                    gsb = npool.tile([P, P], f32, tag="gsb")
                    if got:
                        nc.vector.tensor_copy(gsb[:, :], psum_g[:, :])
                        if l == 0:
                            nc.scalar.copy(hT[:, ws], psum_h[:, :])
                    else:
                        nc.vector.memset(gsb[:, :], 0.0)
                        if l == 0:
                            nc.vector.memset(hT[:, ws], 0.0)

                    psum_o = ps_o.tile([P, P], f32, tag="o")
                    nc.tensor.matmul(psum_o[:, :], wl["Wx"][:, :], gsb[:, :],
                                     start=True, stop=False)
                    nc.tensor.matmul(psum_o[:, :], wl["We"][:, :], hT[:, ws],
                                     start=False, stop=False)
                    nc.tensor.matmul(psum_o[:, :], wl["Wr"][:, :], xt_cur[:, ws],
                                     start=False, stop=True)
                    nc.scalar.activation(xt_next[:, ws], psum_o[:, :],
                                         mybir.ActivationFunctionType.Relu,
                                         bias=wl["br"][:, 0:1])

                    # transpose out^T -> [nodes, feat]
                    psum_t = ps_t.tile([P, P], f32, tag="t")
                    nc.tensor.transpose(psum_t[:, :], xt_next[:, ws], ident_sb[:, :])
                    xn_sb = npool.tile([P, P], f16, tag="xn")
                    nc.scalar.copy(xn_sb[:, :], psum_t[:, :])
                    if l < n_layers - 1:
                        nc.sync.dma_start(agin[l][ws, :], xn_sb[:, :])
                    else:
                        gh = npool.tile([P, GPC], f16, tag="gh")
                        nc.vector.tensor_tensor(
                            gh[:, :], iota_sb[:, :GPC],
                            gid_sb[:, w:w + 1].broadcast_to([P, GPC]),
                            mybir.AluOpType.is_equal)
                        psum_p = ps_p.tile([GPC, OUT], f32, tag="p")
                        nc.tensor.matmul(psum_p[:, :], gh[:, :], xn_sb[:, :],
                                         start=True, stop=True)
                        nc.vector.tensor_tensor(acc_sb[:, :], acc_sb[:, :],
                                                psum_p[:, :],
                                                mybir.AluOpType.add)

                if l < n_layers - 1:
                    nc.gpsimd.collective_compute(
                        "AllGather", mybir.AluOpType.bypass,
                        replica_groups=rep_groups,
                        ins=[agin[l][:, :]],
                        outs=[repl[l][:, :]],
                    )
                    tc.strict_bb_all_engine_barrier()

            nc.sync.dma_start(out_d[:, :], acc_sb[:, :])

    nc.finalize()
    return nc


def _prep_weights(meta, inputs):
    Din, De, EA_W = meta["Din"], meta["De"], meta["EA_W"]
    weights = []
    l = 0
    in_dim = Din
    while f"We{l}" in inputs:
        We = np.asarray(inputs[f"We{l}"], dtype=np.float32)
        be = np.asarray(inputs[f"be{l}"], dtype=np.float32)
        Wr = np.asarray(inputs[f"Wr{l}"], dtype=np.float32)
        br = np.asarray(inputs[f"br{l}"], dtype=np.float32)
        out = We.shape[1]
        Wx = np.zeros((P, out), np.float32)
        Wx[:in_dim] = We[:in_dim]
        WeE = np.zeros((EA_W, out), np.float32)
        WeE[:De] = We[in_dim:in_dim + De]
        WeE[De] = be                       # ones-column applies be exactly
        Wrp = np.zeros((P, out), np.float32)
        Wrp[:in_dim] = Wr
        weights.append(dict(Wx=Wx, We=WeE, Wr=Wrp, br=br.reshape(-1, 1)))
        in_dim = out
        l += 1
    return weights


def kernel(**inputs) -> np.ndarray:
    import sys
    if "/opt/trn_rl_repo" not in sys.path:
        sys.path.insert(0, "/opt/trn_rl_repo")
    from concourse import bass_utils

    x = np.asarray(inputs["x"], dtype=np.float32)
    edge_index = np.asarray(inputs["edge_index"])
    edge_attr = np.asarray(inputs["edge_attr"], dtype=np.float32)
    batch = np.asarray(inputs["batch"])
    n_graphs = int(batch.max()) + 1
    n_graphs = ((n_graphs + N_CORES - 1) // N_CORES) * N_CORES
    n_graphs = max(n_graphs, 64)

    meta, per_core, xrep0 = _plan(x, edge_index, edge_attr, batch, n_graphs)
    weights = _prep_weights(meta, inputs)
    nc = _build(meta, weights)

    iota = np.tile(np.arange(P, dtype=np.float16), (P, 1))
    ident = np.eye(P, dtype=np.float32)
    in_maps = []
    for d in range(N_CORES):
        pc = per_core[d]
        m = dict(xrep0=xrep0, x0T=pc["x0T"], idxs=pc["idx"], dloc=pc["dloc"],
                 ea=pc["ea"], gid=pc["gid"], iota=iota, ident=ident)
        for l, wl in enumerate(weights):
            m[f"Wx{l}"] = wl["Wx"]
            m[f"We{l}"] = wl["We"]
            m[f"Wr{l}"] = wl["Wr"]
            m[f"br{l}"] = wl["br"]
        in_maps.append(m)

    res = bass_utils.run_bass_kernel_spmd(nc, in_maps, core_ids=list(range(N_CORES)))
    kernel.last_results = res
    out = np.concatenate([res.results[d]["out"] for d in range(N_CORES)], axis=0)
    return out

